# revision 16
# baseline (speedup 1.0000x reference)
"""Bi-LSTM Trainium2 kernel: B=64, T=256, D=512, H=512, fp32 I/O.

Sharding: 8 cores = 4 batch groups x 2 directions. Each core runs the full
time recurrence for its 16-sample shard in one direction (the backward
direction is handled by feeding that core a time-reversed input and
un-reversing its output on the host).

On-device layout is fully transposed: hidden/gate dims on SBUF partitions,
batch on the free dim. The recurrent matmul keeps the weight tile as the
stationary (lhsT) operand so the gate output lands transposed in PSUM.

Phase 1 precomputes gx[t] = x_t @ Wx + b for all t into DRAM scratch (f32).
Phase 2 runs the recurrence g = gx[t] + h @ Wh: gx[t] is DVE-copied into
the PSUM bank and the Wh matmuls accumulate on top with start=False, which
removes the per-step identity-inject matmuls (and their PE weight loads)
entirely. Gate columns are host-permuted into half-blocks
[i01 f01 c01 o01 | i23 f23 c23 o23] and the contraction runs as two
half-sweeps (k=0,1 then k=2,3), so the LSTM elementwise tail for the first
half of the hidden state overlaps the tail matmuls of the current step and
the next step's first sweep only waits on the first half-tail.
"""

import sys

for _p in ("/opt/trn_rl_repo",):
    if _p not in sys.path:
        sys.path.append(_p)

import numpy as np
import ml_dtypes

import concourse.bass as bass
import concourse.mybir as mybir
from concourse import tile
from concourse.bass import _add_dep_helper
from concourse.bass_utils import run_bass_kernel_spmd

B, T, D, H = 64, 256, 512, 512
NCORES = 8
GROUPS = 4
BS = B // GROUPS          # batch rows per core
NK = H // 128             # contraction tiles over the hidden dim
NM = (4 * H) // 128       # output tiles over the gate dim
BLK_T = 32                # timesteps per phase-1 block
F32 = mybir.dt.float32
F32R = mybir.dt.float32r
BF16 = mybir.dt.bfloat16

# Gate m-tiles permuted into half-blocks. Original column blocks are
# [i0..3, f0..3, o0..3, c0..3]; on device we use
# [f0,f1, i0,i1, o0,o1, c0,c1,  f2,f3, i2,i3, o2,o3, c2,c3] so each
# half-tail does ONE sigmoid over [f,i,o], its [f,i] part lines up
# elementwise with the [c-state, tanh(cg)] pair tile, and the c tiles
# (which gate the longest chain) finalize last in each half-sweep.
PERM = [4, 5, 0, 1, 8, 9, 12, 13, 6, 7, 2, 3, 10, 11, 14, 15]


def _patch_tail_drain():
    """This image's walrus rejects more than one sync-wait per engine
    instruction (and any wait on a self-loading 4-byte matmul). Tile
    attaches one wait per outstanding semaphore, so split the excess onto
    nofuse nops committed just before the instruction they guard (same
    engine -> identical semantics)."""
    import bass_rust
    from concourse.vector_clock import ScopedClock

    if getattr(tile.TileContext, "_drain_split_patched", False):
        return

    def _drain_and_barrier(self, tick_clock, wait_clock):
        drain_inst = self.nc.sync.drain()
        wait_clock.add_sem_waits(
            drain_inst.ins, ScopedClock({None: tick_clock.global_clock})
        )
        si = drain_inst.ins.sync_info
        if si is not None and len(si.on_wait) > 1:
            waits = list(si.on_wait)
            drain_inst.ins.sync_info = bass_rust.SyncInfo(
                on_wait=waits[:1], on_update=list(si.on_update)
            )
            for i in range(1, len(waits)):
                nop = self.nc.sync.nop(nofuse=True)
                nop.ins.sync_info = bass_rust.SyncInfo(
                    on_wait=waits[i : i + 1], on_update=[]
                )

        self.nc.all_engine_barrier()
        assert self.sems is not None
        popped = self.nc._tile_sem_poison_stack.pop()
        assert popped is self._sem_poison
        self.nc.clear_and_free_semaphores(list(self.sems.allocated().values()))
        self.nc.all_engine_barrier()

    tile.TileContext._drain_and_barrier = _drain_and_barrier

    orig_commit = tile.TileContext._commit_instruction

    def _commit_instruction(self, inst, lazy_reg_writes: bool = True):
        si = getattr(inst, "sync_info", None)
        limit = 0 if isinstance(inst, mybir.InstMatmult) else 1
        if (
            si is not None
            and len(si.on_wait) > limit
            and inst.engine != mybir.EngineType.Unassigned
        ):
            waits = list(si.on_wait)
            keep = waits[len(waits) - limit :] if limit else []
            for w in waits[: len(waits) - limit]:
                nop = mybir.InstNoOp(
                    name=f"I-{self.nc.next_id()}",
                    sync_info=mybir.SyncInfo(on_wait=[w], on_update=[]),
                    bass_nofuse=True,
                    engine=inst.engine,
                )
                orig_commit(self, nop, lazy_reg_writes=False)
            inst.sync_info = mybir.SyncInfo(
                on_wait=keep, on_update=list(si.on_update)
            )
        return orig_commit(self, inst, lazy_reg_writes)

    tile.TileContext._commit_instruction = _commit_instruction
    tile.TileContext._drain_split_patched = True


def build(with_mask: bool = False, t_steps: int = T):
    """Emit the per-core SPMD module."""
    _patch_tail_drain()
    blk_t = min(BLK_T, t_steps)
    nblk = t_steps // blk_t

    nc = bass.Bass("TRN2", target_bir_lowering=False, debug=False,
                   num_devices=NCORES)

    xT = nc.dram_tensor("xT", [D, t_steps * BS], F32R, kind="ExternalInput")
    wx = nc.dram_tensor("wx", [D, 4 * H], F32R, kind="ExternalInput")
    wh = nc.dram_tensor("wh", [H, 4 * H], BF16, kind="ExternalInput")
    bt = nc.dram_tensor("bt", [128, NM], F32, kind="ExternalInput")
    msk = (
        nc.dram_tensor("msk", [t_steps, 128, NK * BS], F32,
                       kind="ExternalInput")
        if with_mask
        else None
    )
    idm = nc.dram_tensor("idm", [128, 128], F32, kind="ExternalInput")
    hout = nc.dram_tensor("hout", [t_steps, 128, NK * BS], BF16,
                          kind="ExternalOutput")
    gxd = nc.dram_tensor("gx_scratch", [128, t_steps, NM * BS], F32,
                         kind="Internal")

    Act = mybir.ActivationFunctionType
    Alu = mybir.AluOpType

    with tile.TileContext(nc) as tc:
        with (
            tc.tile_pool(name="weights", bufs=1) as wpool,
            tc.tile_pool(name="state", bufs=1) as spool,
            tc.tile_pool(name="p1x", bufs=2) as xpool,
            tc.tile_pool(name="p1stg", bufs=2) as stgpool,
            tc.tile_pool(name="p1psum", bufs=2, space="PSUM") as p1ps,
            tc.tile_pool(name="p2psum", bufs=3, space="PSUM") as p2ps,
            tc.tile_pool(name="p2gx", bufs=6) as gxpool,
            tc.tile_pool(name="hstate", bufs=3) as htpool,
            tc.tile_pool(name="p2ew", bufs=2) as ewpool,
        ):
            wxs = wpool.tile([128, NK, 4 * H], F32R)
            nc.gpsimd.dma_start(wxs[:], wx.ap().rearrange("(k p) n -> p k n", p=128))
            whs = wpool.tile([128, NK, 4 * H], BF16)
            nc.gpsimd.dma_start(whs[:], wh.ap().rearrange("(k p) n -> p k n", p=128))
            bts = wpool.tile([128, NM], F32)
            nc.gpsimd.dma_start(bts[:], bt.ap())
            ident = wpool.tile([128, 128], F32)
            nc.gpsimd.dma_start(ident[:], idm.ap())

            hT0 = htpool.tile([128, NK, BS], BF16, tag="hT")
            nc.vector.memset(hT0[:], 0.0)
            hTs = [hT0]
            # per-half [c-state (2 tiles) | tanh(cg) scratch (2 tiles)] so one
            # DVE mult makes both f*c and i*tanh(cg)
            ctgs = [spool.tile([128, 4, BS], F32, name=f"ctg{_h}")
                    for _h in range(2)]
            for _c in ctgs:
                nc.vector.memset(_c[:], 0.0)

            # ---- phase 1 machinery: gx[t] = x_t @ Wx + b ----
            xview = xT.ap().rearrange("(k p) n -> p k n", p=128)
            nfree = blk_t * BS
            p1_tiles: dict = {}
            anchor = [None]
            tail_last: dict = {}

            def p1_mm(blk, m):
                """Matmul part of one m-tile of one phase-1 block."""
                if m == 0:
                    xblk = xpool.tile([128, NK, nfree], F32R, tag="xblk")
                    nc.gpsimd.dma_start(
                        xblk[:], xview[:, :, blk * nfree : (blk + 1) * nfree]
                    )
                    stg = stgpool.tile([128, blk_t, NM, BS], F32, tag="stg")
                    p1_tiles[blk] = (xblk, stg)
                xblk, stg = p1_tiles[blk]
                ps = p1ps.tile([128, nfree], F32, tag="p1ps")
                for k in range(NK):
                    mm = nc.tensor.matmul(
                        ps[:],
                        wxs[:, k, m * 128 : (m + 1) * 128],
                        xblk[:, k, :],
                        start=(k == 0),
                        stop=(k == NK - 1),
                    )
                    if k == 0 and anchor[0] is not None:
                        _add_dep_helper(
                            mm.ins, anchor[0],
                            reason="pin p1 group behind its step",
                        )
                return ps

            def p1_fin(blk, m, ps):
                """Bias add + staging store for one phase-1 m-tile."""
                xblk, stg = p1_tiles[blk]
                psv = ps[:].rearrange("p (t b) -> p t b", b=BS)
                if m % 2 == 0:
                    fi = nc.vector.tensor_scalar(
                        stg[:, :, m, :], psv, bts[:, m : m + 1], None, Alu.add
                    )
                    pin = tail_last.get("dve")
                else:
                    fi = nc.scalar.activation(
                        stg[:, :, m, :], psv, Act.Identity,
                        bias=bts[:, m : m + 1],
                    )
                    pin = tail_last.get("act")
                if pin is not None:
                    _add_dep_helper(fi.ins, pin,
                                    reason="p1 fin after step tail")
                if m == NM - 1:
                    # split the store so early gx loads unblock sooner
                    qt = blk_t // 4 if blk_t % 4 == 0 else blk_t
                    for q in range(blk_t // qt):
                        nc.gpsimd.dma_start(
                            gxd.ap()[
                                :,
                                blk * blk_t + q * qt : blk * blk_t + (q + 1) * qt,
                                :,
                            ],
                            stg[:, q * qt : (q + 1) * qt].rearrange(
                                "p t m b -> p t (m b)"
                            ),
                        )
                    del p1_tiles[blk]

            def p1_group(blk, m):
                p1_fin(blk, m, p1_mm(blk, m))

            # Interleave schedule: block 0 up front; block b's 16 groups
            # spread 1-per-2-steps over the 32 steps before they're needed.
            sched: dict[int, list] = {}
            for b in range(1, nblk):
                if b == 1:
                    slots = [(8 * m) // 5 for m in range(NM)]
                elif b == 2:
                    slots = [16 + 2 * m for m in range(NM)]
                else:
                    slots = [(b - 2) * blk_t + 3 * m for m in range(NM)]
                for m, s in enumerate(slots):
                    sched.setdefault(s, []).append((b, m))
            for m in range(NM):
                p1_group(0, m)

            def wh_mm(G, p, k, stop):
                mm = nc.tensor.matmul(
                    G[:, p, :],
                    whs[:, k, p * 128 : (p + 1) * 128],
                    hTs[0][:, k, :],
                    start=False,
                    stop=stop,
                    skip_group_check=True,
                )
                anchor[0] = mm.ins
                return mm

            def half_tail(t, G, h, hT_new, mtv):
                """Elementwise LSTM cell for hidden half h (0 or 1).

                Gate slices within G for half 0: f=0:2 i=2:4 o=4:6 c=6:8;
                half 1 is the same +8. Writes ctgs[h][:,0:2] and
                hT_new[:, 2h:2h+2].
                """
                o = 8 * h
                s = slice(2 * h, 2 * h + 2)
                ctg = ctgs[h]
                sifo = ewpool.tile([128, 6, BS], F32, tag=f"sifo{h}")
                nc.scalar.activation(sifo[:], G[:, o : o + 6, :], Act.Sigmoid)
                nc.scalar.activation(ctg[:, 2:4, :], G[:, o + 6 : o + 8, :],
                                     Act.Tanh)
                # prod = [sig(f)*c, sig(i)*tanh(cg)] in one op
                prod = ewpool.tile([128, 4, BS], F32, tag=f"prod{h}")
                nc.vector.tensor_tensor(prod[:], sifo[:, 0:4, :], ctg[:],
                                        Alu.mult)
                if with_mask:
                    cn = ewpool.tile([128, 2, BS], F32, tag=f"cn{h}")
                    nc.vector.tensor_tensor(cn[:], prod[:, 0:2, :],
                                            prod[:, 2:4, :], Alu.add)
                    cd = ewpool.tile([128, 2, BS], F32, tag=f"cd{h}")
                    nc.vector.tensor_tensor(cd[:], cn[:], ctg[:, 0:2, :],
                                            Alu.subtract)
                    nc.vector.tensor_tensor(cd[:], cd[:], mtv[:, s, :],
                                            Alu.mult)
                    nc.vector.tensor_tensor(ctg[:, 0:2, :], ctg[:, 0:2, :],
                                            cd[:], Alu.add)
                else:
                    nc.vector.tensor_tensor(ctg[:, 0:2, :], prod[:, 0:2, :],
                                            prod[:, 2:4, :], Alu.add)
                thc = ewpool.tile([128, 2, BS], F32, tag=f"thc{h}")
                tail_last["act"] = nc.scalar.activation(
                    thc[:], ctg[:, 0:2, :], Act.Tanh).ins
                # threshold(o, 0.4): o if o > 0.4 else 0 (og-only, runs early)
                ot = ewpool.tile([128, 2, BS], F32, tag=f"ot{h}")
                nc.vector.scalar_tensor_tensor(
                    ot[:], sifo[:, 4:6, :], 0.4, sifo[:, 4:6, :],
                    Alu.is_gt, Alu.mult
                )
                if with_mask:
                    hn = ewpool.tile([128, 2, BS], F32, tag=f"hn{h}")
                    nc.vector.tensor_tensor(hn[:], ot[:], thc[:], Alu.mult)
                    hd = ewpool.tile([128, 2, BS], F32, tag=f"hd{h}")
                    nc.vector.tensor_tensor(hd[:], hn[:], hTs[0][:, s, :],
                                            Alu.subtract)
                    nc.vector.tensor_tensor(hd[:], hd[:], mtv[:, s, :],
                                            Alu.mult)
                    tail_last["dve"] = nc.vector.tensor_tensor(
                        hT_new[:, s, :], hTs[0][:, s, :], hd[:], Alu.add).ins
                else:
                    tail_last["dve"] = nc.vector.tensor_tensor(
                        hT_new[:, s, :], ot[:], thc[:], Alu.mult).ins

            # ---- phase 2: the recurrence ----
            def load_gx(t):
                gx = gxpool.tile([128, NM, BS], F32, tag="gx")
                nc.sync.dma_start(
                    gx[:],
                    gxd.ap()[:, t, :].rearrange("p (m b) -> p m b", b=BS),
                )
                return gx

            def inject_gx(gx):
                """Open the PSUM group for the next step's gates with a single
                identity matmul that writes gx (start=True also clears the
                bank's pending-zero state so the Wh matmuls accumulate).
                Padded to a full 2KB bank so no other tile shares the
                zero region."""
                Gt = p2ps.tile([128, 512], F32, tag="G")
                mm = nc.tensor.matmul(
                    Gt[:, 0:NM * BS],
                    ident[:],
                    gx[:].rearrange("p m b -> p (m b)"),
                    start=True,
                    stop=False,
                    skip_group_check=True,
                )
                anchor[0] = mm.ins
                return Gt[:, 0 : NM * BS].rearrange("p (m b) -> p m b", b=BS)

            gx_tiles = {tt: load_gx(tt) for tt in range(min(3, t_steps))}
            G_cur = inject_gx(gx_tiles.pop(0))

            for t in range(t_steps):
                if t + 3 < t_steps:
                    gx_tiles[t + 3] = load_gx(t + 3)
                if with_mask:
                    mt = ewpool.tile([128, NK * BS], F32, tag="mt")
                    nc.gpsimd.dma_start(mt[:], msk[t])
                    mtv = mt[:].rearrange("p (k b) -> p k b", b=BS)
                else:
                    mtv = None
                # sweep A: contraction halves k=0,1 for all 16 gate tiles
                for k in (0, 1):
                    for p in range(NM):
                        wh_mm(G_cur, p, k, stop=False)
                # open next step's PSUM gate bank with its gx inject (one
                # PE matmul; bank's last readers are the step t-2 tails)
                if t + 1 < t_steps:
                    G_next = inject_gx(gx_tiles.pop(t + 1))
                # sweep B first half: finalizes gate tiles of half 0
                for p in range(8):
                    for k in (2, 3):
                        wh_mm(G_cur, p, k, stop=(k == 3))
                hT_new = htpool.tile([128, NK, BS], BF16, tag="hT")
                half_tail(t, G_cur, 0, hT_new, mtv)
                # sweep B second half: finalizes gate tiles of half 1
                for p in range(8, NM):
                    for k in (2, 3):
                        wh_mm(G_cur, p, k, stop=(k == 3))
                half_tail(t, G_cur, 1, hT_new, mtv)
                p1w = sched.get(t, ())
                p1ps_live = [p1_mm(b, m) for (b, m) in p1w]
                nc.gpsimd.dma_start(hout[t],
                                    hT_new[:].rearrange("p k b -> p (k b)"))
                for (b, m), ps in zip(p1w, p1ps_live):
                    p1_fin(b, m, ps)
                hTs[0] = hT_new
                if t + 1 < t_steps:
                    G_cur = G_next
    return nc


_BUILD_CACHE: dict = {}


def _get_module(with_mask: bool, t_steps: int = T):
    key = (with_mask, t_steps)
    if key not in _BUILD_CACHE:
        _BUILD_CACHE[key] = build(with_mask, t_steps)
    return _BUILD_CACHE[key]


def _make_in_maps(x, mask, Wf, bf, Wb, bb, with_mask: bool, t_steps: int = T):
    ws = {}
    cperm = np.concatenate([np.arange(m * 128, (m + 1) * 128) for m in PERM])
    for d, (W, bias) in enumerate(((Wf, bf), (Wb, bb))):
        W = np.asarray(W, np.float32)[:, cperm]
        bias = np.asarray(bias, np.float32)[cperm]
        ws[d] = (
            np.ascontiguousarray(W[H:]),                        # wx (x rows)
            np.ascontiguousarray(W[:H].astype(ml_dtypes.bfloat16)),  # wh
            np.ascontiguousarray(bias.reshape(NM, 128).T),
        )
    in_maps = []
    for core in range(NCORES):
        g, d = core // 2, core % 2
        xs = np.asarray(x[g * BS : (g + 1) * BS, :t_steps], np.float32)
        ms = np.asarray(mask[g * BS : (g + 1) * BS, :t_steps], np.float32)
        if d == 1:
            xs = xs[:, ::-1]
            ms = ms[:, ::-1]
        # xT[dd, t*BS + b] = xs[b, t, dd]
        xTv = np.ascontiguousarray(
            xs.transpose(2, 1, 0).reshape(D, t_steps * BS)
        )
        wxv, whv, btv = ws[d]
        m = {"xT": xTv, "wx": wxv, "wh": whv, "bt": btv,
             "idm": np.eye(128, dtype=np.float32)}
        if with_mask:
            m["msk"] = np.ascontiguousarray(
                np.broadcast_to(
                    ms.T[:, None, None, :], (t_steps, 128, NK, BS)
                ).reshape(t_steps, 128, NK * BS)
            )
        in_maps.append(m)
    return in_maps


def _assemble(results, t_steps: int = T):
    out = np.empty((B, t_steps, 2 * H), np.float32)
    for core in range(NCORES):
        g, d = core // 2, core % 2
        h = np.asarray(results[core]["hout"], np.float32)  # [t, 128, NK*BS]
        h = h.reshape(t_steps, 128, NK, BS).transpose(3, 0, 2, 1)  # [b,t,k,p]
        h = h.reshape(BS, t_steps, H)
        if d == 1:
            h = h[:, ::-1]
        out[g * BS : (g + 1) * BS, :, d * H : (d + 1) * H] = h
    return out


def run(x, mask, Wf, bf, Wb, bb, trace=False, t_steps: int = T,
        **spmd_kwargs):
    spmd_kwargs.pop("recur_dt", None)
    spmd_kwargs.pop("p1_dt", None)
    with_mask = not bool(np.all(np.asarray(mask) == 1.0))
    nc = _get_module(with_mask, t_steps)
    in_maps = _make_in_maps(x, mask, Wf, bf, Wb, bb, with_mask, t_steps)
    res = run_bass_kernel_spmd(
        nc, in_maps, list(range(NCORES)), trace=trace, **spmd_kwargs
    )
    return _assemble(res.results, t_steps), res


def kernel(x, mask, Wf, bf, Wb, bb):
    out, _ = run(x, mask, Wf, bf, Wb, bb)
    return out


# revision 17
# speedup vs baseline: 1.0530x; 1.0530x over previous
"""Bi-LSTM Trainium2 kernel: B=64, T=256, D=512, H=512, fp32 I/O.

Sharding: 8 cores = 4 batch groups x 2 directions. Each core runs the full
time recurrence for its 16-sample shard in one direction (the backward
direction is handled by feeding that core a time-reversed input and
un-reversing its output on the host).

On-device layout is fully transposed: hidden/gate dims on SBUF partitions,
batch on the free dim. The recurrent matmul keeps the weight tile as the
stationary (lhsT) operand so the gate output lands transposed in PSUM.

Phase 1 precomputes gx[t] = x_t @ Wx + b for all t into DRAM scratch (f32).
Phase 2 runs the recurrence g = gx[t] + h @ Wh: gx[t] is DVE-copied into
the PSUM bank and the Wh matmuls accumulate on top with start=False, which
removes the per-step identity-inject matmuls (and their PE weight loads)
entirely. Gate columns are host-permuted into half-blocks
[i01 f01 c01 o01 | i23 f23 c23 o23] and the contraction runs as two
half-sweeps (k=0,1 then k=2,3), so the LSTM elementwise tail for the first
half of the hidden state overlaps the tail matmuls of the current step and
the next step's first sweep only waits on the first half-tail.
"""

import sys

for _p in ("/opt/trn_rl_repo",):
    if _p not in sys.path:
        sys.path.append(_p)

import numpy as np
import ml_dtypes

import concourse.bass as bass
import concourse.mybir as mybir
from concourse import tile
from concourse.bass import _add_dep_helper
from concourse.bass_utils import run_bass_kernel_spmd

B, T, D, H = 64, 256, 512, 512
NCORES = 8
GROUPS = 4
BS = B // GROUPS          # batch rows per core
NK = H // 128             # contraction tiles over the hidden dim
NM = (4 * H) // 128       # output tiles over the gate dim
BLK_T = 32                # timesteps per phase-1 block
F32 = mybir.dt.float32
F32R = mybir.dt.float32r
BF16 = mybir.dt.bfloat16

# Gate m-tiles permuted into half-blocks. Original column blocks are
# [i0..3, f0..3, o0..3, c0..3]; on device we use
# [f0,f1, i0,i1, o0,o1, c0,c1,  f2,f3, i2,i3, o2,o3, c2,c3] so each
# half-tail does ONE sigmoid over [f,i,o], its [f,i] part lines up
# elementwise with the [c-state, tanh(cg)] pair tile, and the c tiles
# (which gate the longest chain) finalize last in each half-sweep.
PERM = [4, 5, 0, 1, 8, 9, 12, 13, 6, 7, 2, 3, 10, 11, 14, 15]


def _patch_tail_drain():
    """This image's walrus rejects more than one sync-wait per engine
    instruction (and any wait on a self-loading 4-byte matmul). Tile
    attaches one wait per outstanding semaphore, so split the excess onto
    nofuse nops committed just before the instruction they guard (same
    engine -> identical semantics)."""
    import bass_rust
    from concourse.vector_clock import ScopedClock

    if getattr(tile.TileContext, "_drain_split_patched", False):
        return

    def _drain_and_barrier(self, tick_clock, wait_clock):
        drain_inst = self.nc.sync.drain()
        wait_clock.add_sem_waits(
            drain_inst.ins, ScopedClock({None: tick_clock.global_clock})
        )
        si = drain_inst.ins.sync_info
        if si is not None and len(si.on_wait) > 1:
            waits = list(si.on_wait)
            drain_inst.ins.sync_info = bass_rust.SyncInfo(
                on_wait=waits[:1], on_update=list(si.on_update)
            )
            for i in range(1, len(waits)):
                nop = self.nc.sync.nop(nofuse=True)
                nop.ins.sync_info = bass_rust.SyncInfo(
                    on_wait=waits[i : i + 1], on_update=[]
                )

        self.nc.all_engine_barrier()
        assert self.sems is not None
        popped = self.nc._tile_sem_poison_stack.pop()
        assert popped is self._sem_poison
        self.nc.clear_and_free_semaphores(list(self.sems.allocated().values()))
        self.nc.all_engine_barrier()

    tile.TileContext._drain_and_barrier = _drain_and_barrier

    orig_commit = tile.TileContext._commit_instruction

    def _commit_instruction(self, inst, lazy_reg_writes: bool = True):
        si = getattr(inst, "sync_info", None)
        limit = 0 if isinstance(inst, mybir.InstMatmult) else 1
        if (
            si is not None
            and len(si.on_wait) > limit
            and inst.engine != mybir.EngineType.Unassigned
        ):
            waits = list(si.on_wait)
            keep = waits[len(waits) - limit :] if limit else []
            for w in waits[: len(waits) - limit]:
                nop = mybir.InstNoOp(
                    name=f"I-{self.nc.next_id()}",
                    sync_info=mybir.SyncInfo(on_wait=[w], on_update=[]),
                    bass_nofuse=True,
                    engine=inst.engine,
                )
                orig_commit(self, nop, lazy_reg_writes=False)
            inst.sync_info = mybir.SyncInfo(
                on_wait=keep, on_update=list(si.on_update)
            )
        return orig_commit(self, inst, lazy_reg_writes)

    tile.TileContext._commit_instruction = _commit_instruction
    tile.TileContext._drain_split_patched = True


def build(with_mask: bool = False, t_steps: int = T):
    """Emit the per-core SPMD module."""
    _patch_tail_drain()
    blk_t = min(BLK_T, t_steps)
    nblk = t_steps // blk_t

    nc = bass.Bass("TRN2", target_bir_lowering=False, debug=False,
                   num_devices=NCORES)

    xT = nc.dram_tensor("xT", [D, t_steps * BS], F32R, kind="ExternalInput")
    wx = nc.dram_tensor("wx", [D, 4 * H], F32R, kind="ExternalInput")
    wh = nc.dram_tensor("wh", [H, 4 * H], BF16, kind="ExternalInput")
    bt = nc.dram_tensor("bt", [128, NM], F32, kind="ExternalInput")
    msk = (
        nc.dram_tensor("msk", [t_steps, 128, NK * BS], F32,
                       kind="ExternalInput")
        if with_mask
        else None
    )
    idm = nc.dram_tensor("idm", [128, 128], F32, kind="ExternalInput")
    hout = nc.dram_tensor("hout", [t_steps, 128, NK * BS], BF16,
                          kind="ExternalOutput")
    gxd = nc.dram_tensor("gx_scratch", [128, t_steps, NM * BS], F32,
                         kind="Internal")

    Act = mybir.ActivationFunctionType
    Alu = mybir.AluOpType

    with tile.TileContext(nc) as tc:
        with (
            tc.tile_pool(name="weights", bufs=1) as wpool,
            tc.tile_pool(name="state", bufs=1) as spool,
            tc.tile_pool(name="p1x", bufs=2) as xpool,
            tc.tile_pool(name="p1stg", bufs=2) as stgpool,
            tc.tile_pool(name="p1psum", bufs=2, space="PSUM") as p1ps,
            tc.tile_pool(name="p2psum", bufs=3, space="PSUM") as p2ps,
            tc.tile_pool(name="p2gx", bufs=6) as gxpool,
            tc.tile_pool(name="hstate", bufs=3) as htpool,
            tc.tile_pool(name="p2ew", bufs=2) as ewpool,
        ):
            wxs = wpool.tile([128, NK, 4 * H], F32R)
            nc.gpsimd.dma_start(wxs[:], wx.ap().rearrange("(k p) n -> p k n", p=128))
            whs = wpool.tile([128, NK, 4 * H], BF16)
            nc.gpsimd.dma_start(whs[:], wh.ap().rearrange("(k p) n -> p k n", p=128))
            bts = wpool.tile([128, NM], F32)
            nc.gpsimd.dma_start(bts[:], bt.ap())
            ident = wpool.tile([128, 128], F32)
            nc.gpsimd.dma_start(ident[:], idm.ap())

            hT0 = htpool.tile([128, NK, BS], BF16, tag="hT")
            nc.vector.memset(hT0[:], 0.0)
            hTs = [hT0]
            # per-half [c-state (2 tiles) | tanh(cg) scratch (2 tiles)] so one
            # DVE mult makes both f*c and i*tanh(cg)
            ctgs = [spool.tile([128, 4, BS], F32, name=f"ctg{_h}")
                    for _h in range(2)]
            for _c in ctgs:
                nc.vector.memset(_c[:], 0.0)

            # ---- phase 1 machinery: gx[t] = x_t @ Wx + b ----
            xview = xT.ap().rearrange("(k p) n -> p k n", p=128)
            nfree = blk_t * BS
            p1_tiles: dict = {}
            anchor = [None]
            tail_last: dict = {}

            def p1_mm(blk, m):
                """Matmul part of one m-tile of one phase-1 block."""
                if m == 0:
                    xblk = xpool.tile([128, NK, nfree], F32R, tag="xblk")
                    nc.gpsimd.dma_start(
                        xblk[:], xview[:, :, blk * nfree : (blk + 1) * nfree]
                    )
                    stg = stgpool.tile([128, blk_t, NM, BS], F32, tag="stg")
                    p1_tiles[blk] = (xblk, stg)
                xblk, stg = p1_tiles[blk]
                ps = p1ps.tile([128, nfree], F32, tag="p1ps")
                for k in range(NK):
                    mm = nc.tensor.matmul(
                        ps[:],
                        wxs[:, k, m * 128 : (m + 1) * 128],
                        xblk[:, k, :],
                        start=(k == 0),
                        stop=(k == NK - 1),
                    )
                    if k == 0 and anchor[0] is not None:
                        _add_dep_helper(
                            mm.ins, anchor[0],
                            reason="pin p1 group behind its step",
                        )
                return ps

            def p1_fin(blk, m, ps):
                """Bias add + staging store for one phase-1 m-tile."""
                xblk, stg = p1_tiles[blk]
                psv = ps[:].rearrange("p (t b) -> p t b", b=BS)
                if m % 2 == 0:
                    fi = nc.vector.tensor_scalar(
                        stg[:, :, m, :], psv, bts[:, m : m + 1], None, Alu.add
                    )
                    pin = tail_last.get("dve")
                else:
                    fi = nc.scalar.activation(
                        stg[:, :, m, :], psv, Act.Identity,
                        bias=bts[:, m : m + 1],
                    )
                    pin = tail_last.get("act")
                if pin is not None:
                    _add_dep_helper(fi.ins, pin,
                                    reason="p1 fin after step tail")
                if m == NM - 1:
                    # split the store so early gx loads unblock sooner
                    qt = blk_t // 4 if blk_t % 4 == 0 else blk_t
                    for q in range(blk_t // qt):
                        nc.gpsimd.dma_start(
                            gxd.ap()[
                                :,
                                blk * blk_t + q * qt : blk * blk_t + (q + 1) * qt,
                                :,
                            ],
                            stg[:, q * qt : (q + 1) * qt].rearrange(
                                "p t m b -> p t (m b)"
                            ),
                        )
                    del p1_tiles[blk]

            def p1_group(blk, m):
                p1_fin(blk, m, p1_mm(blk, m))

            # Interleave schedule: block 0 up front; block b's 16 groups
            # spread 1-per-2-steps over the 32 steps before they're needed.
            sched: dict[int, list] = {}
            for b in range(1, nblk):
                if b == 1:
                    slots = [(8 * m) // 5 for m in range(NM)]
                elif b == 2:
                    slots = [16 + 2 * m for m in range(NM)]
                else:
                    slots = [(b - 2) * blk_t + 3 * m for m in range(NM)]
                for m, s in enumerate(slots):
                    sched.setdefault(s, []).append((b, m))
            for m in range(NM):
                p1_group(0, m)

            def wh_mm(G, p, k, stop):
                mm = nc.tensor.matmul(
                    G[:, p, :],
                    whs[:, k, p * 128 : (p + 1) * 128],
                    hTs[0][:, k, :],
                    start=False,
                    stop=stop,
                    skip_group_check=True,
                )
                anchor[0] = mm.ins
                return mm

            def half_tail(t, G, h, hT_new, mtv, pin_act=None):
                """Elementwise LSTM cell for hidden half h (0 or 1).

                Gate slices within G for half 0: f=0:2 i=2:4 o=4:6 c=6:8;
                half 1 is the same +8. Writes ctgs[h][:,0:2] and
                hT_new[:, 2h:2h+2].
                """
                o = 8 * h
                s = slice(2 * h, 2 * h + 2)
                ctg = ctgs[h]
                sifo = ewpool.tile([128, 6, BS], F32, tag=f"sifo{h}")
                si = nc.scalar.activation(sifo[:], G[:, o : o + 6, :],
                                          Act.Sigmoid)
                if pin_act is not None:
                    # keep the greedy scheduler from slotting this half's ACT
                    # ops ahead of the other half's chain-critical tanh(c')
                    _add_dep_helper(si.ins, pin_act,
                                    reason="tail23 ACT after thc01")
                nc.scalar.activation(ctg[:, 2:4, :], G[:, o + 6 : o + 8, :],
                                     Act.Tanh)
                # prod = [sig(f)*c, sig(i)*tanh(cg)] in one op
                prod = ewpool.tile([128, 4, BS], F32, tag=f"prod{h}")
                nc.vector.tensor_tensor(prod[:], sifo[:, 0:4, :], ctg[:],
                                        Alu.mult)
                if with_mask:
                    cn = ewpool.tile([128, 2, BS], F32, tag=f"cn{h}")
                    nc.vector.tensor_tensor(cn[:], prod[:, 0:2, :],
                                            prod[:, 2:4, :], Alu.add)
                    cd = ewpool.tile([128, 2, BS], F32, tag=f"cd{h}")
                    nc.vector.tensor_tensor(cd[:], cn[:], ctg[:, 0:2, :],
                                            Alu.subtract)
                    nc.vector.tensor_tensor(cd[:], cd[:], mtv[:, s, :],
                                            Alu.mult)
                    nc.vector.tensor_tensor(ctg[:, 0:2, :], ctg[:, 0:2, :],
                                            cd[:], Alu.add)
                else:
                    nc.vector.tensor_tensor(ctg[:, 0:2, :], prod[:, 0:2, :],
                                            prod[:, 2:4, :], Alu.add)
                thc = ewpool.tile([128, 2, BS], F32, tag=f"thc{h}")
                tail_last["act"] = nc.scalar.activation(
                    thc[:], ctg[:, 0:2, :], Act.Tanh).ins
                # threshold(o, 0.4): o if o > 0.4 else 0 (og-only, runs early)
                ot = ewpool.tile([128, 2, BS], F32, tag=f"ot{h}")
                nc.vector.scalar_tensor_tensor(
                    ot[:], sifo[:, 4:6, :], 0.4, sifo[:, 4:6, :],
                    Alu.is_gt, Alu.mult
                )
                if with_mask:
                    hn = ewpool.tile([128, 2, BS], F32, tag=f"hn{h}")
                    nc.vector.tensor_tensor(hn[:], ot[:], thc[:], Alu.mult)
                    hd = ewpool.tile([128, 2, BS], F32, tag=f"hd{h}")
                    nc.vector.tensor_tensor(hd[:], hn[:], hTs[0][:, s, :],
                                            Alu.subtract)
                    nc.vector.tensor_tensor(hd[:], hd[:], mtv[:, s, :],
                                            Alu.mult)
                    tail_last["dve"] = nc.vector.tensor_tensor(
                        hT_new[:, s, :], hTs[0][:, s, :], hd[:], Alu.add).ins
                else:
                    tail_last["dve"] = nc.vector.tensor_tensor(
                        hT_new[:, s, :], ot[:], thc[:], Alu.mult).ins

            # ---- phase 2: the recurrence ----
            def load_gx(t):
                gx = gxpool.tile([128, NM, BS], F32, tag="gx")
                nc.sync.dma_start(
                    gx[:],
                    gxd.ap()[:, t, :].rearrange("p (m b) -> p m b", b=BS),
                )
                return gx

            def inject_gx(gx):
                """Open the PSUM group for the next step's gates with a single
                identity matmul that writes gx (start=True also clears the
                bank's pending-zero state so the Wh matmuls accumulate).
                Padded to a full 2KB bank so no other tile shares the
                zero region."""
                Gt = p2ps.tile([128, 512], F32, tag="G")
                mm = nc.tensor.matmul(
                    Gt[:, 0:NM * BS],
                    ident[:],
                    gx[:].rearrange("p m b -> p (m b)"),
                    start=True,
                    stop=False,
                    skip_group_check=True,
                )
                anchor[0] = mm.ins
                return Gt[:, 0 : NM * BS].rearrange("p (m b) -> p m b", b=BS)

            gx_tiles = {tt: load_gx(tt) for tt in range(min(3, t_steps))}
            G_cur = inject_gx(gx_tiles.pop(0))

            for t in range(t_steps):
                if t + 3 < t_steps:
                    gx_tiles[t + 3] = load_gx(t + 3)
                if with_mask:
                    mt = ewpool.tile([128, NK * BS], F32, tag="mt")
                    nc.gpsimd.dma_start(mt[:], msk[t])
                    mtv = mt[:].rearrange("p (k b) -> p k b", b=BS)
                else:
                    mtv = None
                # sweep A: contraction halves k=0,1 for all 16 gate tiles
                for k in (0, 1):
                    for p in range(NM):
                        wh_mm(G_cur, p, k, stop=False)
                # open next step's PSUM gate bank with its gx inject (one
                # PE matmul; bank's last readers are the step t-2 tails)
                if t + 1 < t_steps:
                    G_next = inject_gx(gx_tiles.pop(t + 1))
                # sweep B first half: finalizes gate tiles of half 0
                for p in range(8):
                    for k in (2, 3):
                        wh_mm(G_cur, p, k, stop=(k == 3))
                hT_new = htpool.tile([128, NK, BS], BF16, tag="hT")
                half_tail(t, G_cur, 0, hT_new, mtv)
                thc01_inst = tail_last["act"]
                # sweep B second half: finalizes gate tiles of half 1
                for p in range(8, NM):
                    for k in (2, 3):
                        wh_mm(G_cur, p, k, stop=(k == 3))
                half_tail(t, G_cur, 1, hT_new, mtv, pin_act=thc01_inst)
                p1w = sched.get(t, ())
                p1ps_live = [p1_mm(b, m) for (b, m) in p1w]
                nc.gpsimd.dma_start(hout[t],
                                    hT_new[:].rearrange("p k b -> p (k b)"))
                for (b, m), ps in zip(p1w, p1ps_live):
                    p1_fin(b, m, ps)
                hTs[0] = hT_new
                if t + 1 < t_steps:
                    G_cur = G_next
    return nc


_BUILD_CACHE: dict = {}


def _get_module(with_mask: bool, t_steps: int = T):
    key = (with_mask, t_steps)
    if key not in _BUILD_CACHE:
        _BUILD_CACHE[key] = build(with_mask, t_steps)
    return _BUILD_CACHE[key]


def _make_in_maps(x, mask, Wf, bf, Wb, bb, with_mask: bool, t_steps: int = T):
    ws = {}
    cperm = np.concatenate([np.arange(m * 128, (m + 1) * 128) for m in PERM])
    for d, (W, bias) in enumerate(((Wf, bf), (Wb, bb))):
        W = np.asarray(W, np.float32)[:, cperm]
        bias = np.asarray(bias, np.float32)[cperm]
        ws[d] = (
            np.ascontiguousarray(W[H:]),                        # wx (x rows)
            np.ascontiguousarray(W[:H].astype(ml_dtypes.bfloat16)),  # wh
            np.ascontiguousarray(bias.reshape(NM, 128).T),
        )
    in_maps = []
    for core in range(NCORES):
        g, d = core // 2, core % 2
        xs = np.asarray(x[g * BS : (g + 1) * BS, :t_steps], np.float32)
        ms = np.asarray(mask[g * BS : (g + 1) * BS, :t_steps], np.float32)
        if d == 1:
            xs = xs[:, ::-1]
            ms = ms[:, ::-1]
        # xT[dd, t*BS + b] = xs[b, t, dd]
        xTv = np.ascontiguousarray(
            xs.transpose(2, 1, 0).reshape(D, t_steps * BS)
        )
        wxv, whv, btv = ws[d]
        m = {"xT": xTv, "wx": wxv, "wh": whv, "bt": btv,
             "idm": np.eye(128, dtype=np.float32)}
        if with_mask:
            m["msk"] = np.ascontiguousarray(
                np.broadcast_to(
                    ms.T[:, None, None, :], (t_steps, 128, NK, BS)
                ).reshape(t_steps, 128, NK * BS)
            )
        in_maps.append(m)
    return in_maps


def _assemble(results, t_steps: int = T):
    out = np.empty((B, t_steps, 2 * H), np.float32)
    for core in range(NCORES):
        g, d = core // 2, core % 2
        h = np.asarray(results[core]["hout"], np.float32)  # [t, 128, NK*BS]
        h = h.reshape(t_steps, 128, NK, BS).transpose(3, 0, 2, 1)  # [b,t,k,p]
        h = h.reshape(BS, t_steps, H)
        if d == 1:
            h = h[:, ::-1]
        out[g * BS : (g + 1) * BS, :, d * H : (d + 1) * H] = h
    return out


def run(x, mask, Wf, bf, Wb, bb, trace=False, t_steps: int = T,
        **spmd_kwargs):
    spmd_kwargs.pop("recur_dt", None)
    spmd_kwargs.pop("p1_dt", None)
    with_mask = not bool(np.all(np.asarray(mask) == 1.0))
    nc = _get_module(with_mask, t_steps)
    in_maps = _make_in_maps(x, mask, Wf, bf, Wb, bb, with_mask, t_steps)
    res = run_bass_kernel_spmd(
        nc, in_maps, list(range(NCORES)), trace=trace, **spmd_kwargs
    )
    return _assemble(res.results, t_steps), res


def kernel(x, mask, Wf, bf, Wb, bb):
    out, _ = run(x, mask, Wf, bf, Wb, bb)
    return out


# revision 19
# speedup vs baseline: 1.0581x; 1.0049x over previous
"""Bi-LSTM Trainium2 kernel: B=64, T=256, D=512, H=512, fp32 I/O.

Sharding: 8 cores = 4 batch groups x 2 directions. Each core runs the full
time recurrence for its 16-sample shard in one direction (the backward
direction is handled by feeding that core a time-reversed input and
un-reversing its output on the host).

On-device layout is fully transposed: hidden/gate dims on SBUF partitions,
batch on the free dim. The recurrent matmul keeps the weight tile as the
stationary (lhsT) operand so the gate output lands transposed in PSUM.

Phase 1 precomputes gx[t] = x_t @ Wx + b for all t into DRAM scratch (f32).
Phase 2 runs the recurrence g = gx[t] + h @ Wh: gx[t] is DVE-copied into
the PSUM bank and the Wh matmuls accumulate on top with start=False, which
removes the per-step identity-inject matmuls (and their PE weight loads)
entirely. Gate columns are host-permuted into half-blocks
[i01 f01 c01 o01 | i23 f23 c23 o23] and the contraction runs as two
half-sweeps (k=0,1 then k=2,3), so the LSTM elementwise tail for the first
half of the hidden state overlaps the tail matmuls of the current step and
the next step's first sweep only waits on the first half-tail.
"""

import sys

for _p in ("/opt/trn_rl_repo",):
    if _p not in sys.path:
        sys.path.append(_p)

import numpy as np
import ml_dtypes

import concourse.bass as bass
import concourse.mybir as mybir
from concourse import tile
from concourse.bass import _add_dep_helper
from concourse.bass_utils import run_bass_kernel_spmd

B, T, D, H = 64, 256, 512, 512
NCORES = 8
GROUPS = 4
BS = B // GROUPS          # batch rows per core
NK = H // 128             # contraction tiles over the hidden dim
NM = (4 * H) // 128       # output tiles over the gate dim
BLK_T = 32                # timesteps per phase-1 block
F32 = mybir.dt.float32
F32R = mybir.dt.float32r
BF16 = mybir.dt.bfloat16

# Gate m-tiles permuted into half-blocks. Original column blocks are
# [i0..3, f0..3, o0..3, c0..3]; on device we use
# [f0,f1, i0,i1, o0,o1, c0,c1,  f2,f3, i2,i3, o2,o3, c2,c3] so each
# half-tail does ONE sigmoid over [f,i,o], its [f,i] part lines up
# elementwise with the [c-state, tanh(cg)] pair tile, and the c tiles
# (which gate the longest chain) finalize last in each half-sweep.
PERM = [4, 5, 0, 1, 8, 9, 12, 13, 6, 7, 2, 3, 10, 11, 14, 15]


def _patch_tail_drain():
    """This image's walrus rejects more than one sync-wait per engine
    instruction (and any wait on a self-loading 4-byte matmul). Tile
    attaches one wait per outstanding semaphore, so split the excess onto
    nofuse nops committed just before the instruction they guard (same
    engine -> identical semantics)."""
    import bass_rust
    from concourse.vector_clock import ScopedClock

    if getattr(tile.TileContext, "_drain_split_patched", False):
        return

    def _drain_and_barrier(self, tick_clock, wait_clock):
        drain_inst = self.nc.sync.drain()
        wait_clock.add_sem_waits(
            drain_inst.ins, ScopedClock({None: tick_clock.global_clock})
        )
        si = drain_inst.ins.sync_info
        if si is not None and len(si.on_wait) > 1:
            waits = list(si.on_wait)
            drain_inst.ins.sync_info = bass_rust.SyncInfo(
                on_wait=waits[:1], on_update=list(si.on_update)
            )
            for i in range(1, len(waits)):
                nop = self.nc.sync.nop(nofuse=True)
                nop.ins.sync_info = bass_rust.SyncInfo(
                    on_wait=waits[i : i + 1], on_update=[]
                )

        self.nc.all_engine_barrier()
        assert self.sems is not None
        popped = self.nc._tile_sem_poison_stack.pop()
        assert popped is self._sem_poison
        self.nc.clear_and_free_semaphores(list(self.sems.allocated().values()))
        self.nc.all_engine_barrier()

    tile.TileContext._drain_and_barrier = _drain_and_barrier

    orig_commit = tile.TileContext._commit_instruction

    def _commit_instruction(self, inst, lazy_reg_writes: bool = True):
        si = getattr(inst, "sync_info", None)
        limit = 0 if isinstance(inst, mybir.InstMatmult) else 1
        if (
            si is not None
            and len(si.on_wait) > limit
            and inst.engine != mybir.EngineType.Unassigned
        ):
            waits = list(si.on_wait)
            keep = waits[len(waits) - limit :] if limit else []
            for w in waits[: len(waits) - limit]:
                nop = mybir.InstNoOp(
                    name=f"I-{self.nc.next_id()}",
                    sync_info=mybir.SyncInfo(on_wait=[w], on_update=[]),
                    bass_nofuse=True,
                    engine=inst.engine,
                )
                orig_commit(self, nop, lazy_reg_writes=False)
            inst.sync_info = mybir.SyncInfo(
                on_wait=keep, on_update=list(si.on_update)
            )
        return orig_commit(self, inst, lazy_reg_writes)

    tile.TileContext._commit_instruction = _commit_instruction
    tile.TileContext._drain_split_patched = True


def build(with_mask: bool = False, t_steps: int = T):
    """Emit the per-core SPMD module."""
    _patch_tail_drain()
    blk_t = min(BLK_T, t_steps)
    nblk = t_steps // blk_t

    nc = bass.Bass("TRN2", target_bir_lowering=False, debug=False,
                   num_devices=NCORES)

    xT = nc.dram_tensor("xT", [D, t_steps * BS], F32R, kind="ExternalInput")
    wx = nc.dram_tensor("wx", [D, 4 * H], F32R, kind="ExternalInput")
    wh = nc.dram_tensor("wh", [H, 4 * H], BF16, kind="ExternalInput")
    bt = nc.dram_tensor("bt", [128, NM], F32, kind="ExternalInput")
    msk = (
        nc.dram_tensor("msk", [t_steps, 128, NK * BS], F32,
                       kind="ExternalInput")
        if with_mask
        else None
    )
    idm = nc.dram_tensor("idm", [128, 128], F32, kind="ExternalInput")
    hout = nc.dram_tensor("hout", [t_steps, 128, NK * BS], BF16,
                          kind="ExternalOutput")
    gxd = nc.dram_tensor("gx_scratch", [128, t_steps, NM * BS], F32,
                         kind="Internal")

    Act = mybir.ActivationFunctionType
    Alu = mybir.AluOpType

    with tile.TileContext(nc) as tc:
        with (
            tc.tile_pool(name="weights", bufs=1) as wpool,
            tc.tile_pool(name="state", bufs=1) as spool,
            tc.tile_pool(name="p1x", bufs=2) as xpool,
            tc.tile_pool(name="p1stg", bufs=2) as stgpool,
            tc.tile_pool(name="p1psum", bufs=2, space="PSUM") as p1ps,
            tc.tile_pool(name="p2psum", bufs=3, space="PSUM") as p2ps,
            tc.tile_pool(name="p2gx", bufs=12) as gxpool,
            tc.tile_pool(name="hstate", bufs=3) as htpool,
            tc.tile_pool(name="p2ew", bufs=2) as ewpool,
        ):
            wxs = wpool.tile([128, NK, 4 * H], F32R)
            nc.gpsimd.dma_start(wxs[:], wx.ap().rearrange("(k p) n -> p k n", p=128))
            whs = wpool.tile([128, NK, 4 * H], BF16)
            nc.gpsimd.dma_start(whs[:], wh.ap().rearrange("(k p) n -> p k n", p=128))
            bts = wpool.tile([128, NM], F32)
            nc.gpsimd.dma_start(bts[:], bt.ap())
            ident = wpool.tile([128, 128], F32)
            nc.gpsimd.dma_start(ident[:], idm.ap())

            hT0 = htpool.tile([128, NK, BS], BF16, tag="hT")
            nc.vector.memset(hT0[:], 0.0)
            hTs = [hT0]
            # per-half [c-state (2 tiles) | tanh(cg) scratch (2 tiles)] so one
            # DVE mult makes both f*c and i*tanh(cg)
            ctgs = [spool.tile([128, 4, BS], F32, name=f"ctg{_h}")
                    for _h in range(2)]
            for _c in ctgs:
                nc.vector.memset(_c[:], 0.0)

            # ---- phase 1 machinery: gx[t] = x_t @ Wx + b ----
            xview = xT.ap().rearrange("(k p) n -> p k n", p=128)
            nfree = blk_t * BS
            p1_tiles: dict = {}
            anchor = [None]
            tail_last: dict = {}

            def p1_mm(blk, m):
                """Matmul part of one m-tile of one phase-1 block."""
                if m == 0:
                    xblk = xpool.tile([128, NK, nfree], F32R, tag="xblk")
                    nc.gpsimd.dma_start(
                        xblk[:], xview[:, :, blk * nfree : (blk + 1) * nfree]
                    )
                    stg = stgpool.tile([128, blk_t, NM, BS], F32, tag="stg")
                    p1_tiles[blk] = (xblk, stg)
                xblk, stg = p1_tiles[blk]
                ps = p1ps.tile([128, nfree], F32, tag="p1ps")
                for k in range(NK):
                    mm = nc.tensor.matmul(
                        ps[:],
                        wxs[:, k, m * 128 : (m + 1) * 128],
                        xblk[:, k, :],
                        start=(k == 0),
                        stop=(k == NK - 1),
                    )
                    if k == 0 and anchor[0] is not None:
                        _add_dep_helper(
                            mm.ins, anchor[0],
                            reason="pin p1 group behind its step",
                        )
                return ps

            def p1_fin(blk, m, ps):
                """Bias add + staging store for one phase-1 m-tile."""
                xblk, stg = p1_tiles[blk]
                psv = ps[:].rearrange("p (t b) -> p t b", b=BS)
                if m % 2 == 0:
                    fi = nc.vector.tensor_scalar(
                        stg[:, :, m, :], psv, bts[:, m : m + 1], None, Alu.add
                    )
                    pin = tail_last.get("dve")
                else:
                    fi = nc.scalar.activation(
                        stg[:, :, m, :], psv, Act.Identity,
                        bias=bts[:, m : m + 1],
                    )
                    pin = tail_last.get("act")
                if pin is not None:
                    _add_dep_helper(fi.ins, pin,
                                    reason="p1 fin after step tail")
                if m == NM - 1:
                    # split the store so early gx loads unblock sooner
                    qt = blk_t // 4 if blk_t % 4 == 0 else blk_t
                    for q in range(blk_t // qt):
                        nc.gpsimd.dma_start(
                            gxd.ap()[
                                :,
                                blk * blk_t + q * qt : blk * blk_t + (q + 1) * qt,
                                :,
                            ],
                            stg[:, q * qt : (q + 1) * qt].rearrange(
                                "p t m b -> p t (m b)"
                            ),
                        )
                    del p1_tiles[blk]

            def p1_group(blk, m):
                p1_fin(blk, m, p1_mm(blk, m))

            # Interleave schedule: block 0 up front; block b's 16 groups
            # spread 1-per-2-steps over the 32 steps before they're needed.
            sched: dict[int, list] = {}
            for b in range(1, nblk):
                if b == 1:
                    slots = [(3 * m) // 2 for m in range(NM)]
                elif b == 2:
                    slots = [16 + 2 * m for m in range(NM)]
                else:
                    slots = [(b - 2) * blk_t + 3 * m for m in range(NM)]
                for m, s in enumerate(slots):
                    sched.setdefault(s, []).append((b, m))
            for m in range(NM):
                p1_group(0, m)

            def wh_mm(G, p, k, stop):
                mm = nc.tensor.matmul(
                    G[:, p, :],
                    whs[:, k, p * 128 : (p + 1) * 128],
                    hTs[0][:, k, :],
                    start=False,
                    stop=stop,
                    skip_group_check=True,
                )
                anchor[0] = mm.ins
                return mm

            def half_tail(t, G, h, hT_new, mtv, pin_act=None):
                """Elementwise LSTM cell for hidden half h (0 or 1).

                Gate slices within G for half 0: f=0:2 i=2:4 o=4:6 c=6:8;
                half 1 is the same +8. Writes ctgs[h][:,0:2] and
                hT_new[:, 2h:2h+2].
                """
                o = 8 * h
                s = slice(2 * h, 2 * h + 2)
                ctg = ctgs[h]
                sifo = ewpool.tile([128, 6, BS], F32, tag=f"sifo{h}")
                si = nc.scalar.activation(sifo[:], G[:, o : o + 6, :],
                                          Act.Sigmoid)
                if pin_act is not None:
                    # keep the greedy scheduler from slotting this half's ACT
                    # ops ahead of the other half's chain-critical tanh(c')
                    _add_dep_helper(si.ins, pin_act,
                                    reason="tail23 ACT after thc01")
                nc.scalar.activation(ctg[:, 2:4, :], G[:, o + 6 : o + 8, :],
                                     Act.Tanh)
                # prod = [sig(f)*c, sig(i)*tanh(cg)] in one op
                prod = ewpool.tile([128, 4, BS], F32, tag=f"prod{h}")
                nc.vector.tensor_tensor(prod[:], sifo[:, 0:4, :], ctg[:],
                                        Alu.mult)
                if with_mask:
                    cn = ewpool.tile([128, 2, BS], F32, tag=f"cn{h}")
                    nc.vector.tensor_tensor(cn[:], prod[:, 0:2, :],
                                            prod[:, 2:4, :], Alu.add)
                    cd = ewpool.tile([128, 2, BS], F32, tag=f"cd{h}")
                    nc.vector.tensor_tensor(cd[:], cn[:], ctg[:, 0:2, :],
                                            Alu.subtract)
                    nc.vector.tensor_tensor(cd[:], cd[:], mtv[:, s, :],
                                            Alu.mult)
                    nc.vector.tensor_tensor(ctg[:, 0:2, :], ctg[:, 0:2, :],
                                            cd[:], Alu.add)
                else:
                    nc.vector.tensor_tensor(ctg[:, 0:2, :], prod[:, 0:2, :],
                                            prod[:, 2:4, :], Alu.add)
                thc = ewpool.tile([128, 2, BS], F32, tag=f"thc{h}")
                tail_last["act"] = nc.scalar.activation(
                    thc[:], ctg[:, 0:2, :], Act.Tanh).ins
                # threshold(o, 0.4): o if o > 0.4 else 0 (og-only, runs early)
                ot = ewpool.tile([128, 2, BS], F32, tag=f"ot{h}")
                nc.vector.scalar_tensor_tensor(
                    ot[:], sifo[:, 4:6, :], 0.4, sifo[:, 4:6, :],
                    Alu.is_gt, Alu.mult
                )
                if with_mask:
                    hn = ewpool.tile([128, 2, BS], F32, tag=f"hn{h}")
                    nc.vector.tensor_tensor(hn[:], ot[:], thc[:], Alu.mult)
                    hd = ewpool.tile([128, 2, BS], F32, tag=f"hd{h}")
                    nc.vector.tensor_tensor(hd[:], hn[:], hTs[0][:, s, :],
                                            Alu.subtract)
                    nc.vector.tensor_tensor(hd[:], hd[:], mtv[:, s, :],
                                            Alu.mult)
                    tail_last["dve"] = nc.vector.tensor_tensor(
                        hT_new[:, s, :], hTs[0][:, s, :], hd[:], Alu.add).ins
                else:
                    tail_last["dve"] = nc.vector.tensor_tensor(
                        hT_new[:, s, :], ot[:], thc[:], Alu.mult).ins

            # ---- phase 2: the recurrence ----
            def load_gx(t):
                gx = gxpool.tile([128, NM, BS], F32, tag="gx")
                nc.sync.dma_start(
                    gx[:],
                    gxd.ap()[:, t, :].rearrange("p (m b) -> p m b", b=BS),
                )
                return gx

            def inject_gx(gx):
                """Open the PSUM group for the next step's gates with a single
                identity matmul that writes gx (start=True also clears the
                bank's pending-zero state so the Wh matmuls accumulate).
                Padded to a full 2KB bank so no other tile shares the
                zero region."""
                Gt = p2ps.tile([128, 512], F32, tag="G")
                mm = nc.tensor.matmul(
                    Gt[:, 0:NM * BS],
                    ident[:],
                    gx[:].rearrange("p m b -> p (m b)"),
                    start=True,
                    stop=False,
                    skip_group_check=True,
                )
                anchor[0] = mm.ins
                return Gt[:, 0 : NM * BS].rearrange("p (m b) -> p m b", b=BS)

            gx_tiles = {tt: load_gx(tt) for tt in range(min(6, t_steps))}
            G_cur = inject_gx(gx_tiles.pop(0))

            for t in range(t_steps):
                if t + 6 < t_steps:
                    gx_tiles[t + 6] = load_gx(t + 6)
                if with_mask:
                    mt = ewpool.tile([128, NK * BS], F32, tag="mt")
                    nc.gpsimd.dma_start(mt[:], msk[t])
                    mtv = mt[:].rearrange("p (k b) -> p k b", b=BS)
                else:
                    mtv = None
                # sweep A: contraction halves k=0,1 for all 16 gate tiles
                for k in (0, 1):
                    for p in range(NM):
                        wh_mm(G_cur, p, k, stop=False)
                # open next step's PSUM gate bank with its gx inject (one
                # PE matmul; bank's last readers are the step t-2 tails)
                if t + 1 < t_steps:
                    G_next = inject_gx(gx_tiles.pop(t + 1))
                # sweep B first half: finalizes gate tiles of half 0
                for p in range(8):
                    for k in (2, 3):
                        wh_mm(G_cur, p, k, stop=(k == 3))
                hT_new = htpool.tile([128, NK, BS], BF16, tag="hT")
                half_tail(t, G_cur, 0, hT_new, mtv)
                thc01_inst = tail_last["act"]
                # sweep B second half: finalizes gate tiles of half 1
                for p in range(8, NM):
                    for k in (2, 3):
                        wh_mm(G_cur, p, k, stop=(k == 3))
                half_tail(t, G_cur, 1, hT_new, mtv, pin_act=thc01_inst)
                p1w = sched.get(t, ())
                p1ps_live = [p1_mm(b, m) for (b, m) in p1w]
                nc.gpsimd.dma_start(hout[t],
                                    hT_new[:].rearrange("p k b -> p (k b)"))
                for (b, m), ps in zip(p1w, p1ps_live):
                    p1_fin(b, m, ps)
                hTs[0] = hT_new
                if t + 1 < t_steps:
                    G_cur = G_next
    return nc


_BUILD_CACHE: dict = {}


def _get_module(with_mask: bool, t_steps: int = T):
    key = (with_mask, t_steps)
    if key not in _BUILD_CACHE:
        _BUILD_CACHE[key] = build(with_mask, t_steps)
    return _BUILD_CACHE[key]


def _make_in_maps(x, mask, Wf, bf, Wb, bb, with_mask: bool, t_steps: int = T):
    ws = {}
    cperm = np.concatenate([np.arange(m * 128, (m + 1) * 128) for m in PERM])
    for d, (W, bias) in enumerate(((Wf, bf), (Wb, bb))):
        W = np.asarray(W, np.float32)[:, cperm]
        bias = np.asarray(bias, np.float32)[cperm]
        ws[d] = (
            np.ascontiguousarray(W[H:]),                        # wx (x rows)
            np.ascontiguousarray(W[:H].astype(ml_dtypes.bfloat16)),  # wh
            np.ascontiguousarray(bias.reshape(NM, 128).T),
        )
    in_maps = []
    for core in range(NCORES):
        g, d = core // 2, core % 2
        xs = np.asarray(x[g * BS : (g + 1) * BS, :t_steps], np.float32)
        ms = np.asarray(mask[g * BS : (g + 1) * BS, :t_steps], np.float32)
        if d == 1:
            xs = xs[:, ::-1]
            ms = ms[:, ::-1]
        # xT[dd, t*BS + b] = xs[b, t, dd]
        xTv = np.ascontiguousarray(
            xs.transpose(2, 1, 0).reshape(D, t_steps * BS)
        )
        wxv, whv, btv = ws[d]
        m = {"xT": xTv, "wx": wxv, "wh": whv, "bt": btv,
             "idm": np.eye(128, dtype=np.float32)}
        if with_mask:
            m["msk"] = np.ascontiguousarray(
                np.broadcast_to(
                    ms.T[:, None, None, :], (t_steps, 128, NK, BS)
                ).reshape(t_steps, 128, NK * BS)
            )
        in_maps.append(m)
    return in_maps


def _assemble(results, t_steps: int = T):
    out = np.empty((B, t_steps, 2 * H), np.float32)
    for core in range(NCORES):
        g, d = core // 2, core % 2
        h = np.asarray(results[core]["hout"], np.float32)  # [t, 128, NK*BS]
        h = h.reshape(t_steps, 128, NK, BS).transpose(3, 0, 2, 1)  # [b,t,k,p]
        h = h.reshape(BS, t_steps, H)
        if d == 1:
            h = h[:, ::-1]
        out[g * BS : (g + 1) * BS, :, d * H : (d + 1) * H] = h
    return out


def run(x, mask, Wf, bf, Wb, bb, trace=False, t_steps: int = T,
        **spmd_kwargs):
    spmd_kwargs.pop("recur_dt", None)
    spmd_kwargs.pop("p1_dt", None)
    with_mask = not bool(np.all(np.asarray(mask) == 1.0))
    nc = _get_module(with_mask, t_steps)
    in_maps = _make_in_maps(x, mask, Wf, bf, Wb, bb, with_mask, t_steps)
    res = run_bass_kernel_spmd(
        nc, in_maps, list(range(NCORES)), trace=trace, **spmd_kwargs
    )
    return _assemble(res.results, t_steps), res


def kernel(x, mask, Wf, bf, Wb, bb):
    out, _ = run(x, mask, Wf, bf, Wb, bb)
    return out
